# revision 1
# baseline (speedup 1.0000x reference)
"""Dense-CRF mean-field inference on 8 Trainium2 NeuronCores.

Math restructuring (validated numerically against the jax reference):
  - Kb + Kg share weight 1.0 -> single kernel matrix K = exp(-.5 d2_b) + exp(-.5 d2_g).
  - The Potts 3x3 conv update is  upd[c] = boxsum3(S) - boxsum3(comb[c]) with
    S = sum_c comb[c]; the S part is class-independent so softmax drops it:
        out = softmax(input + UPDATE_FACTOR * boxsum3(comb[c])).
    The UPDATE_FACTOR (3.0) is folded into K via exp(x + ln 3).
  - Spatial sigma 5 -> K decays fast with |dy|; rows further than ~20 image rows
    from the output pixel contribute < 1e-5 relative.  Each core keeps a
    41-block (5248 px) band of K rows resident in SBUF: blocks within +-6 rows
    in fp32, the rest fp16 (validated: l2 rel err 2.9e-5 vs fp32-exact 2.2e-5).
  - -0.5*||fi-fj||^2 is computed by ONE matmul per kernel via augmented
    features: G=[y,x,-.5|s|^2,1,r,g,b,-.5|c|^2,1], H=[y,x,1,-.5|s|^2,r,g,b,1,-.5|c|^2];
    gaussian = rows 0:4, bilateral = rows 0:9.
  - Each core computes comb for 14 image rows (its 12 + 1 halo row each side,
    edge rows duplicated via clamped features) so the 3x3 conv is local.
    One AllGather of the new per-core probabilities per iteration.

Sharding: core r owns output image rows [12r, 12r+12); K band = global
128-px blocks [9r-16, 9r+25) (zero-K padding outside the image).
"""

import os
import sys

import numpy as np

for _p in ("/opt/trn_rl_repo",):
    if _p not in sys.path and os.path.isdir(_p):
        sys.path.insert(0, _p)

H = 96
W = 96
C = 5
N = H * W                      # 9216
NCORES = 8
RPC = H // NCORES              # 12 image rows per core
NLOC = (RPC + 2) * W           # 1344 extended-output pixels (14 rows)
NMID = RPC * W                 # 1152 owned pixels
BLK = 128
NBLK = 41                      # K band m-blocks per core
BAND_LO = -16                  # band start, in global blocks, relative to 9r
F32_LO, F32_HI = 12, 29        # band-local block range kept in fp32 (+-4 rows)
N32 = F32_HI - F32_LO          # 21 fp32 blocks
N16 = NBLK - N32               # 20 fp16 blocks
GBLK = N // BLK                # 72 global blocks
PADBLK = 16                    # padding blocks each side of flat_padded
FPW = (GBLK + 2 * PADBLK) * C  # flat_padded free width = 520
CH = 448                       # matvec/exp n-chunk (fits one PSUM bank)
NCH = 3
ITERS = 5
LN3 = float(np.log(3.0))
NEG = -1.0e30                  # kills exp() for out-of-image padding blocks

_CACHED_NC = None


def _near(i):
    return F32_LO <= i < F32_HI


def _k16_idx(i):
    return i if i < F32_LO else i - N32


def _build_module():
    import concourse.bass as bass
    import concourse.bacc as bacc
    import concourse.tile as tile
    from concourse import mybir
    from concourse.masks import make_identity

    f32 = mybir.dt.float32
    f16 = mybir.dt.float16
    u32 = mybir.dt.uint32
    EXP = mybir.ActivationFunctionType.Exp
    COPY = mybir.ActivationFunctionType.Copy

    nc = bacc.Bacc("TRN2", target_bir_lowering=False, debug=False,
                   num_devices=NCORES)

    g_dram = nc.dram_tensor("g_feats", [9, NBLK * BLK], f32, kind="ExternalInput")
    h_dram = nc.dram_tensor("h_feats", [9, NLOC], f32, kind="ExternalInput")
    ipp_dram = nc.dram_tensor("inp_pp", [BLK, GBLK * C], f32, kind="ExternalInput")
    icn_dram = nc.dram_tensor("inp_cn", [C, NMID], f32, kind="ExternalInput")
    boff_dram = nc.dram_tensor("band_off", [1, 1], u32, kind="ExternalInput")
    kg32_dram = nc.dram_tensor("kg32", [BLK, N32 * NCH * CH], f32,
                               kind="ExternalInput")
    kg16_dram = nc.dram_tensor("kg16", [BLK, N16 * NCH * CH], f16,
                               kind="ExternalInput")
    out_dram = nc.dram_tensor("out_loc", [BLK, (NMID // BLK) * C], f32,
                              kind="ExternalOutput")

    def bcast_inner(ap, n):
        return bass.AP(tensor=ap.tensor, offset=ap.offset, ap=[*ap.ap, [0, n]])

    with tile.TileContext(nc) as tc:
        with tc.tile_pool(name="singles", bufs=1) as singles, \
             tc.tile_pool(name="warmps", bufs=1, space="PSUM") as warmpool, \
             tc.tile_pool(name="dram", bufs=1, space="DRAM") as dram:

            # ---- long-lived SBUF state ----
            k32 = singles.tile([BLK, N32, NCH * CH], f32, name="k32")
            k16 = singles.tile([BLK, N16, NCH * CH], f16, name="k16")
            flat_pad = singles.tile([BLK, FPW], f32, name="flat_pad")
            h_sb = singles.tile([9, NLOC], f32, name="h_sb")
            ipp_sb = singles.tile([BLK, GBLK * C], f32, name="ipp_sb")
            icn_sb = singles.tile([C, NMID], f32, name="icn_sb")
            ident = singles.tile([BLK, BLK], f32, name="ident")
            boff_sb = singles.tile([1, 1], u32, name="boff_sb")
            ln3_sb = singles.tile([BLK, 1], f32, name="ln3_sb")
            nc.vector.memset(ln3_sb, LN3)
            # HAM warm-keeper: dummy matmuls that fill PE-idle windows so the
            # activity monitor keeps the PE clock at 2.4 GHz (it halves the
            # clock after ~3.4us of idle).  ~426 ns each (fp32 512-col).
            warm_ps = warmpool.tile([1, 512], f32, name="warm_ps")

            def warm(n):
                for _ in range(n):
                    nc.tensor.matmul(warm_ps, ident[:, 0:1], k32[:, 0, 0:512],
                                     start=True, stop=True)

            ag_in = dram.tile([BLK, (NMID // BLK) * C], f32, name="ag_in")
            ag_out = dram.tile([BLK * NCORES, (NMID // BLK) * C], f32, name="ag_out")

            nc.sync.dma_start(out=h_sb, in_=h_dram[:, :])
            nc.sync.dma_start(out=ipp_sb, in_=ipp_dram[:, :])
            nc.sync.dma_start(out=icn_sb, in_=icn_dram[:, :])
            nc.sync.dma_start(out=boff_sb, in_=boff_dram[:, :])
            make_identity(nc, ident)
            nc.vector.memset(flat_pad, 0.0)

            # band offset register (elements into flat_pad) = 45 * core_id
            boff_regs = nc.alloc_registers("boff_regs",
                                           engines=(mybir.EngineType.DVE,))
            nc.regs_load(boff_regs, boff_sb[0:1, 0:1])
            off_sv = nc.snap(boff_regs, donate=True, min_val=0,
                             max_val=(NCORES - 1) * 9 * C)

            # ---- phase 1: build K band ----
            # Bilateral part on device (input-dependent); the gaussian part is
            # input-independent so the host ships it precomputed (kg32/kg16)
            # and we just add it.
            with tc.tile_pool(name="gstage", bufs=3) as gpool, \
                 tc.tile_pool(name="kgstage", bufs=3) as kgpool, \
                 tc.tile_pool(name="bpsum", bufs=2, space="PSUM") as bppool:
                for i in range(NBLK):
                    gt = gpool.tile([9, BLK], f32, tag="gt")
                    nc.sync.dma_start(out=gt, in_=g_dram[:, i * BLK:(i + 1) * BLK])
                    if _near(i):
                        kdst = k32[:, i - F32_LO, :]
                        kdt = f32
                        j = i - F32_LO
                        kg_src = kg32_dram[:, j * NCH * CH:(j + 1) * NCH * CH]
                    else:
                        kdst = k16[:, _k16_idx(i), :]
                        kdt = f16
                        j = _k16_idx(i)
                        kg_src = kg16_dram[:, j * NCH * CH:(j + 1) * NCH * CH]
                    kg = kgpool.tile([BLK, NCH * CH], kdt, tag="kg")
                    nc.sync.dma_start(out=kg, in_=kg_src)
                    pb = bppool.tile([BLK, NCH, 512], f32, tag="pb")
                    for nb in range(NCH):
                        hs = h_sb[:, nb * CH:(nb + 1) * CH]
                        nc.tensor.matmul(pb[:, nb, 0:CH], gt[0:9, :], hs[0:9, :],
                                         start=True, stop=True)
                    kv = kdst.rearrange("p (a c) -> p a c", c=CH)
                    nc.scalar.activation(out=kv, in_=pb[:, :, 0:CH], func=EXP,
                                         bias=ln3_sb)
                    nc.vector.tensor_add(kdst, kdst, kg)
                warm(12)

            # ---- helpers ----
            def softmax_pp(pool, u_pp, mb, tag):
                """u_pp: [128, mb*C] logits, pixel-partition layout -> probs."""
                v = u_pp.rearrange("p (a c) -> p a c", c=C)
                mx = pool.tile([BLK, mb], f32, tag=f"{tag}_mx")
                nc.vector.tensor_reduce(out=mx, in_=v,
                                        axis=mybir.AxisListType.X,
                                        op=mybir.AluOpType.max)
                e = pool.tile([BLK, mb * C], f32, tag=f"{tag}_e")
                ev = e.rearrange("p (a c) -> p a c", c=C)
                nc.vector.tensor_sub(ev, v, bcast_inner(mx, C))
                nc.scalar.activation(out=e, in_=e, func=EXP)
                s = pool.tile([BLK, mb], f32, tag=f"{tag}_s")
                nc.vector.tensor_reduce(out=s, in_=ev,
                                        axis=mybir.AxisListType.X,
                                        op=mybir.AluOpType.add)
                nc.vector.reciprocal(out=s, in_=s)
                fl = pool.tile([BLK, mb * C], f32, tag=f"{tag}_fl")
                nc.vector.tensor_mul(fl.rearrange("p (a c) -> p a c", c=C), ev,
                                     bcast_inner(s, C))
                return fl

            # ---- phase 2: initial flat = softmax(input) ----
            with tc.tile_pool(name="init", bufs=1) as ipool:
                fl0 = softmax_pp(ipool, ipp_sb, GBLK, "sm0")
                nc.vector.tensor_copy(
                    out=flat_pad[:, PADBLK * C:(PADBLK + GBLK) * C], in_=fl0)

            # ---- phase 3: iterations ----
            with tc.tile_pool(name="iter", bufs=1) as wpool, \
                 tc.tile_pool(name="band", bufs=2) as bpool, \
                 tc.tile_pool(name="smx", bufs=2) as spool, \
                 tc.tile_pool(name="ipsum", bufs=2, space="PSUM") as ippool:
                for it in range(ITERS):
                    band32 = bpool.tile([BLK, NBLK * C], f32, tag="band32")
                    nc.vector.tensor_copy(
                        out=band32, in_=flat_pad[:, bass.ds(off_sv, NBLK * C)])
                    band16 = bpool.tile([BLK, NBLK * C], f16, tag="band16")
                    nc.vector.tensor_copy(out=band16, in_=band32)

                    # matvec: comb[c, n] = sum_m K[m, n] * flat[c, m]
                    pv = ippool.tile([C, NCH, 512], f32, tag="pv", bufs=1)
                    for nb in range(NCH):
                        for i in range(NBLK):
                            if _near(i):
                                lhs = band32[:, i * C:(i + 1) * C]
                                kt = k32[:, i - F32_LO, nb * CH:(nb + 1) * CH]
                            else:
                                lhs = band16[:, i * C:(i + 1) * C]
                                kt = k16[:, _k16_idx(i), nb * CH:(nb + 1) * CH]
                            nc.tensor.matmul(pv[:, nb, 0:CH], lhs, kt,
                                             start=(i == 0), stop=(i == NBLK - 1))
                    warm(20)
                    comb = wpool.tile([C, NLOC], f32, tag="comb")
                    nc.scalar.activation(
                        out=comb.rearrange("p (a c) -> p a c", c=CH),
                        in_=pv[:, :, 0:CH], func=COPY)

                    # 3x3 box sum: x-pass into t1 (all 14 rows), edge-replicated
                    t1 = wpool.tile([C, NLOC], f32, tag="t1")
                    nc.vector.tensor_add(t1[:, 1:NLOC - 1], comb[:, 0:NLOC - 2],
                                         comb[:, 2:NLOC])
                    nc.vector.tensor_add(t1[:, 1:NLOC - 1], t1[:, 1:NLOC - 1],
                                         comb[:, 1:NLOC - 1])
                    t1r = t1.rearrange("p (row x) -> p row x", x=W)
                    cbr = comb.rearrange("p (row x) -> p row x", x=W)
                    # x = 0 column: 2*c[0] + c[1]
                    nc.vector.tensor_add(t1r[:, :, 0:1], cbr[:, :, 0:1],
                                         cbr[:, :, 1:2])
                    nc.vector.tensor_add(t1r[:, :, 0:1], t1r[:, :, 0:1],
                                         cbr[:, :, 0:1])
                    # x = W-1 column: c[W-2] + 2*c[W-1]
                    nc.vector.tensor_add(t1r[:, :, W - 1:W], cbr[:, :, W - 2:W - 1],
                                         cbr[:, :, W - 1:W])
                    nc.vector.tensor_add(t1r[:, :, W - 1:W], t1r[:, :, W - 1:W],
                                         cbr[:, :, W - 1:W])
                    # y-pass (middle 12 rows) + input logits
                    u = wpool.tile([C, NMID], f32, tag="u")
                    nc.vector.tensor_add(u, t1[:, 0:NMID], t1[:, 2 * W:NLOC])
                    nc.vector.tensor_add(u, u, t1[:, W:NMID + W])
                    nc.vector.tensor_add(u, u, icn_sb)

                    # transpose U [5, 1152] -> pixel-partition [128, 9*5]
                    u_pp = spool.tile([BLK, (NMID // BLK) * C], f32, tag="u_pp")
                    for kb in range(NMID // BLK):
                        pt = ippool.tile([BLK, C], f32, tag="pt")
                        nc.tensor.transpose(pt, u[:, kb * BLK:(kb + 1) * BLK],
                                            ident[0:C, 0:C])
                        nc.vector.tensor_copy(out=u_pp[:, kb * C:(kb + 1) * C],
                                              in_=pt)

                    flat_l = softmax_pp(spool, u_pp, NMID // BLK, "smx")
                    if it < ITERS - 1:
                        warm(42)

                    if it < ITERS - 1:
                        nc.sync.dma_start(out=ag_in, in_=flat_l)
                        nc.gpsimd.collective_compute(
                            "AllGather",
                            mybir.AluOpType.bypass,
                            replica_groups=[list(range(NCORES))],
                            ins=[ag_in.opt()],
                            outs=[ag_out.opt()],
                        )
                        nc.sync.dma_start(
                            out=flat_pad[:, PADBLK * C:(PADBLK + GBLK) * C]
                            .rearrange("p (r j) -> p r j", r=NCORES),
                            in_=ag_out.rearrange("(r p) j -> p r j", p=BLK))
                    else:
                        nc.sync.dma_start(out=out_dram[:, :], in_=flat_l)

    nc.compile()
    return nc


def _host_inputs(input_tensor, reference_tensor):
    logits = np.ascontiguousarray(
        np.asarray(input_tensor, dtype=np.float32)[0].reshape(C, N))
    ref = np.asarray(reference_tensor, dtype=np.float32)[0]  # [3, 96, 96]

    yy, xx = np.meshgrid(np.arange(H, dtype=np.float32),
                         np.arange(W, dtype=np.float32), indexing="ij")
    Y = (yy / 5.0).reshape(N)
    X = (xx / 5.0).reshape(N)
    RGB = (ref / 0.5).reshape(3, N)
    s2 = -0.5 * (Y * Y + X * X)
    c2 = -0.5 * (RGB * RGB).sum(axis=0)
    ones = np.ones(N, np.float32)

    # G (band / m side) and H (output / n side) augmented features
    G_all = np.stack([Y, X, s2, ones, RGB[0], RGB[1], RGB[2], c2, ones])
    H_all = np.stack([Y, X, ones, s2, RGB[0], RGB[1], RGB[2], ones, c2])

    # input in pixel-partition layout [128, 72*5]
    ipp = np.ascontiguousarray(
        logits.reshape(C, GBLK, BLK).transpose(2, 1, 0).reshape(BLK, GBLK * C))

    # gaussian kernel tables: 3*exp(-(dy^2+dx^2)/50), folded update factor 3
    dtab = np.exp(-(np.arange(-(H - 1), H) ** 2) / 50.0).astype(np.float64)
    gx3 = (3.0 * dtab).astype(np.float32)
    gy1 = dtab.astype(np.float32)
    yy_all = (np.arange(N) // W).astype(np.int64)
    xx_all = (np.arange(N) % W).astype(np.int64)

    def kg_for_core(r, yn, xn):
        """[NBLK, 128, 1344] gaussian kernel values for core r's band."""
        kg = np.zeros((NBLK, BLK, NLOC), np.float32)
        for i in range(NBLK):
            gb = 9 * r + BAND_LO + i
            if 0 <= gb < GBLK:
                pm = np.arange(gb * BLK, (gb + 1) * BLK)
                A = gy1[yy_all[pm][:, None] - yn[None, :] + H - 1]
                B = gx3[xx_all[pm][:, None] - xn[None, :] + H - 1]
                kg[i] = A * B
        return kg

    in_maps = []
    kg_interior = None
    for r in range(NCORES):
        g = np.zeros((9, NBLK * BLK), np.float32)
        g[2, :] = NEG
        for i in range(NBLK):
            gb = 9 * r + BAND_LO + i
            if 0 <= gb < GBLK:
                g[:, i * BLK:(i + 1) * BLK] = G_all[:, gb * BLK:(gb + 1) * BLK]
        yext = np.clip(np.arange(RPC * r - 1, RPC * (r + 1) + 1), 0, H - 1)
        hpix = (yext[:, None] * W + np.arange(W)[None, :]).reshape(-1)
        h = np.ascontiguousarray(H_all[:, hpix])
        icn = np.ascontiguousarray(
            logits.reshape(C, H, W)[:, RPC * r:RPC * (r + 1), :].reshape(C, NMID))
        # gaussian part of K (interior cores share one array)
        if 2 <= r <= 5:
            if kg_interior is None:
                kg_interior = kg_for_core(r, yy_all[hpix], xx_all[hpix])
            kg = kg_interior
        else:
            kg = kg_for_core(r, yy_all[hpix], xx_all[hpix])
        near_idx = list(range(F32_LO, F32_HI))
        far_idx = [i for i in range(NBLK) if not _near(i)]
        far_idx = sorted(far_idx, key=_k16_idx)
        kg32 = np.ascontiguousarray(
            kg[near_idx].transpose(1, 0, 2).reshape(BLK, N32 * NLOC))
        kg16 = np.ascontiguousarray(
            kg[far_idx].transpose(1, 0, 2).reshape(BLK, N16 * NLOC)
        ).astype(np.float16)
        in_maps.append({
            "g_feats": g,
            "h_feats": h,
            "inp_pp": ipp,
            "inp_cn": icn,
            "band_off": np.array([[9 * C * r]], np.uint32),
            "kg32": kg32,
            "kg16": kg16,
        })
    return in_maps


def _assemble(results):
    out = np.empty((C, N), np.float32)
    for r in range(NCORES):
        blk = results[r]["out_loc"].reshape(BLK, NMID // BLK, C)
        out[:, NMID * r:NMID * (r + 1)] = (
            blk.transpose(2, 1, 0).reshape(C, NMID))
    return out.reshape(1, C, H, W)


def _get_nc():
    global _CACHED_NC
    if _CACHED_NC is None:
        _CACHED_NC = _build_module()
    return _CACHED_NC


def run(input_tensor, reference_tensor, trace=False):
    from concourse.bass_utils import run_bass_kernel_spmd
    nc = _get_nc()
    in_maps = _host_inputs(input_tensor, reference_tensor)
    res = run_bass_kernel_spmd(nc, in_maps, core_ids=list(range(NCORES)),
                               trace=trace)
    return _assemble(res.results), res


def kernel(input_tensor, reference_tensor):
    out, _ = run(input_tensor, reference_tensor, trace=False)
    return out



# revision 4
# speedup vs baseline: 2.9667x; 2.9667x over previous
"""Dense-CRF mean-field inference on 8 Trainium2 NeuronCores (v2).

Restructure vs v1 (855937 ns baseline):
  - K = 3*(Kb + Kg) built as K = exp(d2y + d2c + ln3) * Tv + SP3 where
    Tv = exp(-dx^2/50) is an exact Toeplitz x-table (3 variants, 128-px
    blocks repeat mod 3) and SP3 = 3*gy*gx is the host-precomputed
    spatial gaussian (fp16, streamed from HBM during phase 1).
  - d2y + d2color come from ONE bf16 matmul (1 cyc/row vs fp32's 4)
    with hi/lo-split compensated features (19 rows) -> fp32-accurate.
  - K stored fp16; matvec in fp16 (1 cyc/row).  Accuracy is restored by
    a compensated matvec: flat is gathered in fp32, split into fp16
    hi+lo, and K@lo is accumulated over the near band (+-4 rows) only.
  - Per-chunk bands: 3 row-aligned n-chunks (5/5/4 rows), each with its
    own +-20-row m-band (34 blocks) -> 102 hi + 30 lo matmuls/iter.
  - No fat fp32 warm matmuls; early dummy AllGather absorbs the cold
    collective-ring setup (first real gather then runs ~14us warm).
  - Own-block matmuls of the next iteration are emitted before the
    far-block ones so the PE works through them during the gather.

Sharding: core r owns output image rows [12r, 12r+12); K band = global
128-px blocks [9r-16, 9r+25) (zero-K padding outside the image).
Validated on host: rel_err 4.9e-4 (gate 2e-2), see validate.py.
"""

import os
import sys

import numpy as np

for _p in ("/opt/trn_rl_repo",):
    if _p not in sys.path and os.path.isdir(_p):
        sys.path.insert(0, _p)

H = 96
W = 96
C = 5
N = H * W                      # 9216
NCORES = 8
RPC = H // NCORES              # 12 image rows per core
NLOC = (RPC + 2) * W           # 1344 extended-output pixels (14 rows)
NMID = RPC * W                 # 1152 owned pixels
BLK = 128
GBLK = N // BLK                # 72 global blocks
ITERS = 5
LN3 = float(np.log(3.0))
NEG = -1.0e30

# chunk c: ext-local rows [lo_r, hi_r) relative to 12r; ap = cols
CHUNKS = [(-1, 4, 480), (4, 9, 480), (9, 13, 384)]
T_BAND = 20                    # band margin rows
T_COMP = 4                     # compensated (K@flat_lo) margin rows
BAND = [(-16, 18), (-12, 22), (-9, 25)]     # rel block ranges (T=20)
COMP = [(-4, 6), (0, 10), (3, 13)]          # rel block ranges (+-4)
BAND_LO = -16                  # union band start (blocks, rel to 9r)
NBLK_U = 41                    # union band size in blocks
OWN_LO, OWN_HI = 16, 25        # own blocks in band-local coords
NBLKC = [hi - lo for lo, hi in BAND]        # [34, 34, 34]
KW = sum(nb * ap for nb, (_, _, ap) in zip(NBLKC, CHUNKS))   # 45696
CBASE = [0, 34 * 480, 34 * 480 + 34 * 480]  # K col base per chunk
PADBLK = 16
FPW = (GBLK + 2 * PADBLK) * C  # flat_pad cols = 520
NFEAT = 19
ACTB = 4                       # blocks per activation/vector batch

_CACHED_NC = None


def _build_module():
    import concourse.bass as bass
    import concourse.bacc as bacc
    import concourse.tile as tile
    from concourse import mybir
    from concourse.masks import make_identity

    f32 = mybir.dt.float32
    f16 = mybir.dt.float16
    bf16 = mybir.dt.bfloat16
    u32 = mybir.dt.uint32
    EXP = mybir.ActivationFunctionType.Exp
    COPY = mybir.ActivationFunctionType.Copy

    nc = bacc.Bacc("TRN2", target_bir_lowering=False, debug=False,
                   num_devices=NCORES)

    g_dram = nc.dram_tensor("g_feats", [NFEAT, NBLK_U * BLK], bf16,
                            kind="ExternalInput")
    h_dram = nc.dram_tensor("h_feats", [NFEAT, NLOC], bf16,
                            kind="ExternalInput")
    sp3_dram = nc.dram_tensor("sp3", [BLK, KW], f16, kind="ExternalInput")
    tv_dram = nc.dram_tensor("tv", [BLK, 7 * W], f16, kind="ExternalInput")
    ipp_dram = nc.dram_tensor("inp_pp", [BLK, GBLK * C], f32,
                              kind="ExternalInput")
    icn_dram = nc.dram_tensor("inp_cn", [C, NMID], f32, kind="ExternalInput")
    boff_dram = nc.dram_tensor("band_off", [1, 1], u32, kind="ExternalInput")
    out_dram = nc.dram_tensor("out_loc", [BLK, (NMID // BLK) * C], f32,
                              kind="ExternalOutput")

    def bcast_inner(ap, n):
        return bass.AP(tensor=ap.tensor, offset=ap.offset, ap=[*ap.ap, [0, n]])

    with tile.TileContext(nc) as tc:
        with tc.tile_pool(name="singles", bufs=1) as singles, \
             tc.tile_pool(name="dram", bufs=1, space="DRAM") as dram:

            # ---- long-lived SBUF state ----
            kt = [singles.tile([BLK, NBLKC[ci] * CHUNKS[ci][2]], f16,
                               name=f"k{ci}") for ci in range(3)]
            h_sb = singles.tile([NFEAT, NLOC], bf16, name="h_sb")
            g_sb = singles.tile([NFEAT, NBLK_U * BLK], bf16, name="g_sb")
            tv_sb = singles.tile([BLK, 7 * W], f16, name="tv_sb")
            flat_pad = singles.tile([BLK, FPW], f32, name="flat_pad")
            own32 = singles.tile([BLK, 9 * C], f32, name="own32")
            own_hi = singles.tile([BLK, 9 * C], f16, name="own_hi")
            own_lo = singles.tile([BLK, 9 * C], f16, name="own_lo")
            band_hi = singles.tile([BLK, NBLK_U * C], f16, name="band_hi")
            band_lo = singles.tile([BLK, NBLK_U * C], f16, name="band_lo")
            ipp_sb = singles.tile([BLK, GBLK * C], f32, name="ipp_sb")
            icn_sb = singles.tile([C, NMID], f32, name="icn_sb")
            ident = singles.tile([BLK, BLK], f32, name="ident")
            boff_sb = singles.tile([1, 1], u32, name="boff_sb")
            ln3_sb = singles.tile([BLK, 1], f32, name="ln3_sb")
            comb_t1 = singles.tile([C, NLOC], f32, name="t1")
            comb_sb = singles.tile([C, NLOC], f32, name="comb_sb")
            u_cn = singles.tile([C, NMID], f32, name="u_cn")
            u_pp = singles.tile([BLK, 9 * C], f32, name="u_pp")
            nc.vector.memset(ln3_sb, LN3)

            ag_in = dram.tile([BLK, 9 * C], f32, name="ag_in")
            ag_out = nc.dram_tensor("ag_out", [BLK * NCORES, 9 * C], f32,
                                    addr_space="Shared")
            wg_in = dram.tile([BLK, 1], f32, name="wg_in")
            wg_out = nc.dram_tensor("wg_out", [BLK * NCORES, 1], f32,
                                    addr_space="Shared")

            nc.sync.dma_start(out=h_sb, in_=h_dram[:, :])
            nc.sync.dma_start(out=g_sb, in_=g_dram[:, :])
            nc.sync.dma_start(out=tv_sb, in_=tv_dram[:, :])
            nc.sync.dma_start(out=ipp_sb, in_=ipp_dram[:, :])
            nc.sync.dma_start(out=icn_sb, in_=icn_dram[:, :])
            nc.sync.dma_start(out=boff_sb, in_=boff_dram[:, :])
            make_identity(nc, ident)
            nc.vector.memset(flat_pad, 0.0)

            # warm-up collective: absorbs the cold ring-setup cost (~40us)
            # concurrently with phase 1 so real gathers run warm (~14us).
            nc.sync.dma_start(out=wg_in, in_=ipp_dram[:, 0:1])
            nc.gpsimd.collective_compute(
                "AllGather", mybir.AluOpType.bypass,
                replica_groups=[list(range(NCORES))],
                ins=[wg_in.opt()], outs=[wg_out[:, :]],
            )

            boff_regs = nc.alloc_registers("boff_regs",
                                           engines=(mybir.EngineType.DVE,))
            nc.regs_load(boff_regs, boff_sb[0:1, 0:1])
            off_sv = nc.snap(boff_regs, donate=True, min_val=0,
                             max_val=(NCORES - 1) * 9 * C)

            # ---- helpers ----
            def softmax_pp(pool, u_ppv, mb, tag, out=None):
                v = u_ppv.rearrange("p (a c) -> p a c", c=C)
                mx = pool.tile([BLK, mb], f32, tag=f"{tag}_mx")
                nc.vector.tensor_reduce(out=mx, in_=v,
                                        axis=mybir.AxisListType.X,
                                        op=mybir.AluOpType.max)
                e = pool.tile([BLK, mb * C], f32, tag=f"{tag}_e")
                ev = e.rearrange("p (a c) -> p a c", c=C)
                nc.vector.tensor_sub(ev, v, bcast_inner(mx, C))
                nc.scalar.activation(out=e, in_=e, func=EXP)
                s = pool.tile([BLK, mb], f32, tag=f"{tag}_s")
                nc.vector.tensor_reduce(out=s, in_=ev,
                                        axis=mybir.AxisListType.X,
                                        op=mybir.AluOpType.add)
                nc.vector.reciprocal(out=s, in_=s)
                if out is None:
                    out = pool.tile([BLK, mb * C], f32, tag=f"{tag}_fl")
                nc.vector.tensor_mul(out.rearrange("p (a c) -> p a c", c=C),
                                     ev, bcast_inner(s, C))
                return out

            # ---- phase 2: initial flat = softmax(input), replicated ----
            with tc.tile_pool(name="init", bufs=1) as ipool:
                fl0 = softmax_pp(ipool, ipp_sb, GBLK, "sm0")
                nc.vector.tensor_copy(
                    out=flat_pad[:, PADBLK * C:(PADBLK + GBLK) * C], in_=fl0)
                nc.vector.tensor_copy(out=own32,
                                      in_=fl0[:, bass.ds(off_sv, 9 * C)])
                nc.vector.tensor_copy(out=own_hi, in_=own32)
                nc.vector.tensor_sub(own_lo, own32, own_hi)

            # ---- phase 1: build K band (fp16) ----
            with tc.tile_pool(name="sp3p", bufs=2) as sp3pool, \
                 tc.tile_pool(name="ep", bufs=2) as epool, \
                 tc.tile_pool(name="p1ps", bufs=2, space="PSUM") as p1pool:
                for ci in range(3):
                    lo_b, hi_b = BAND[ci]
                    nb, ap = NBLKC[ci], CHUNKS[ci][2]
                    e0 = (CHUNKS[ci][0] + 1) * W
                    sp3_sb = sp3pool.tile([BLK, 34 * 480], f16, tag="sp3")
                    nc.sync.dma_start(
                        out=sp3_sb[:, 0:nb * ap],
                        in_=sp3_dram[:, CBASE[ci]:CBASE[ci] + nb * ap])
                    ktv = kt[ci].rearrange("p (j a) -> p j a", a=ap)
                    sp3v = sp3_sb[:, 0:nb * ap].rearrange(
                        "p (j a) -> p j a", a=ap)
                    for j0 in range(0, nb, ACTB):
                        nj = min(ACTB, nb - j0)
                        pb = p1pool.tile([BLK, ACTB, 512], f32, tag="pb")
                        for jj in range(nj):
                            bi = lo_b - BAND_LO + j0 + jj
                            nc.tensor.matmul(
                                pb[:, jj, 0:ap],
                                g_sb[:, bi * BLK:(bi + 1) * BLK],
                                h_sb[:, e0:e0 + ap],
                                start=True, stop=True)
                        eb = epool.tile([BLK, ACTB * 480], f16, tag="eb")
                        ebv = eb[:, 0:nj * ap].rearrange(
                            "p (j a) -> p j a", a=ap)
                        nc.scalar.activation(out=ebv, in_=pb[:, 0:nj, 0:ap],
                                             func=EXP, bias=ln3_sb)
                        # K = E * Tv  (x-Toeplitz; variant = block idx mod 3)
                        v0 = (lo_b + j0) % 3
                        tv_ap = bass.AP(
                            tensor=tv_sb.tensor,
                            offset=tv_sb[:, v0 * W:v0 * W + 1].offset,
                            ap=[tv_sb.ap[0], [W, nj], [0, ap // W], [1, W]])
                        nc.vector.tensor_mul(
                            ktv[:, j0:j0 + nj, :].rearrange(
                                "p j (r x) -> p j r x", x=W),
                            ebv.rearrange("p j (r x) -> p j r x", x=W),
                            tv_ap)
                        # K += SP3 (precomputed 3*gy*gx, fp16)
                        nc.vector.tensor_add(ktv[:, j0:j0 + nj, :],
                                             ktv[:, j0:j0 + nj, :],
                                             sp3v[:, j0:j0 + nj, :])

            # ---- phase 3: iterations ----
            with tc.tile_pool(name="smx", bufs=2) as spool, \
                 tc.tile_pool(name="ipsum", bufs=1, space="PSUM") as ippool, \
                 tc.tile_pool(name="tpsum", bufs=2, space="PSUM") as tppool:
                pv = [ippool.tile([C, 512], f32, tag=f"pv{ci}",
                                  name=f"pv{ci}") for ci in range(3)]

                def src(bi, hi):
                    t = (own_hi if hi else own_lo) if OWN_LO <= bi < OWN_HI \
                        else (band_hi if hi else band_lo)
                    o = (bi - OWN_LO) if OWN_LO <= bi < OWN_HI else bi
                    return t[:, o * C:(o + 1) * C]

                for it in range(ITERS):
                    # own-block matmuls first: run during the gather
                    started = [False, False, False]
                    for ci in range(3):
                        lo_b, hi_b = BAND[ci]
                        cl, ch = COMP[ci]
                        ap = CHUNKS[ci][2]
                        ktv = kt[ci].rearrange("p (j a) -> p j a", a=ap)
                        for b in range(max(lo_b, 0), min(hi_b, 9)):
                            bi = b - BAND_LO
                            j = b - lo_b
                            nc.tensor.matmul(pv[ci][:, 0:ap], src(bi, True),
                                             ktv[:, j, :],
                                             start=not started[ci],
                                             stop=False)
                            started[ci] = True
                            if cl <= b < ch:
                                nc.tensor.matmul(pv[ci][:, 0:ap],
                                                 src(bi, False), ktv[:, j, :],
                                                 start=False, stop=False)
                    # band tiles from gathered flat (stalls until scatter)
                    nc.vector.tensor_copy(
                        out=band_hi,
                        in_=flat_pad[:, bass.ds(off_sv, NBLK_U * C)])
                    nc.vector.tensor_sub(
                        band_lo, flat_pad[:, bass.ds(off_sv, NBLK_U * C)],
                        band_hi)
                    # far blocks + x-pass per chunk
                    for ci in range(3):
                        lo_b, hi_b = BAND[ci]
                        cl, ch = COMP[ci]
                        lo_r, hi_r, ap = CHUNKS[ci]
                        nrows = hi_r - lo_r
                        ktv = kt[ci].rearrange("p (j a) -> p j a", a=ap)
                        far = [b for b in range(lo_b, hi_b)
                               if not (0 <= b < 9)]
                        for n, b in enumerate(far):
                            bi = b - BAND_LO
                            j = b - lo_b
                            last = (n == len(far) - 1)
                            lo_here = cl <= b < ch
                            nc.tensor.matmul(pv[ci][:, 0:ap], src(bi, True),
                                             ktv[:, j, :], start=False,
                                             stop=last and not lo_here)
                            if lo_here:
                                nc.tensor.matmul(pv[ci][:, 0:ap],
                                                 src(bi, False), ktv[:, j, :],
                                                 start=False, stop=last)
                        # PSUM -> SBUF (scalar), then x-pass of the box sum
                        e0 = (lo_r + 1) * W
                        cb = comb_sb[:, e0:e0 + ap]
                        nc.scalar.activation(out=cb, in_=pv[ci][:, 0:ap],
                                             func=COPY)
                        t1c = comb_t1[:, e0:e0 + ap]
                        nc.vector.tensor_add(t1c[:, 1:ap - 1], cb[:, 0:ap - 2],
                                             cb[:, 2:ap])
                        nc.vector.tensor_add(t1c[:, 1:ap - 1], t1c[:, 1:ap - 1],
                                             cb[:, 1:ap - 1])
                        t1r = t1c.rearrange("p (r x) -> p r x", x=W)
                        cbr = cb.rearrange("p (r x) -> p r x", x=W)
                        nc.vector.tensor_add(t1r[:, :, 0:1], cbr[:, :, 0:1],
                                             cbr[:, :, 1:2])
                        nc.vector.tensor_add(t1r[:, :, 0:1], t1r[:, :, 0:1],
                                             cbr[:, :, 0:1])
                        nc.vector.tensor_add(t1r[:, :, W - 1:W],
                                             cbr[:, :, W - 2:W - 1],
                                             cbr[:, :, W - 1:W])
                        nc.vector.tensor_add(t1r[:, :, W - 1:W],
                                             t1r[:, :, W - 1:W],
                                             cbr[:, :, W - 1:W])
                        # y-pass A after chunk 1 (u rows 0..7)
                        if ci == 1:
                            nc.vector.tensor_add(u_cn[:, 0:768],
                                                 comb_t1[:, 0:768],
                                                 comb_t1[:, W:768 + W])
                            nc.vector.tensor_add(u_cn[:, 0:768],
                                                 u_cn[:, 0:768],
                                                 comb_t1[:, 2 * W:768 + 2 * W])
                            nc.vector.tensor_add(u_cn[:, 0:768],
                                                 u_cn[:, 0:768],
                                                 icn_sb[:, 0:768])
                            for kb in range(6):
                                pt = tppool.tile([BLK, C], f32, tag="pt")
                                nc.tensor.transpose(
                                    pt, u_cn[:, kb * BLK:(kb + 1) * BLK],
                                    ident[0:C, 0:C])
                                nc.vector.tensor_copy(
                                    out=u_pp[:, kb * C:(kb + 1) * C], in_=pt)
                    # y-pass B (u rows 8..11) + remaining transposes
                    nc.vector.tensor_add(u_cn[:, 768:NMID],
                                         comb_t1[:, 768:NMID],
                                         comb_t1[:, 768 + W:NMID + W])
                    nc.vector.tensor_add(u_cn[:, 768:NMID], u_cn[:, 768:NMID],
                                         comb_t1[:, 768 + 2 * W:NMID + 2 * W])
                    nc.vector.tensor_add(u_cn[:, 768:NMID], u_cn[:, 768:NMID],
                                         icn_sb[:, 768:NMID])
                    for kb in range(6, 9):
                        pt = tppool.tile([BLK, C], f32, tag="pt")
                        nc.tensor.transpose(pt,
                                            u_cn[:, kb * BLK:(kb + 1) * BLK],
                                            ident[0:C, 0:C])
                        nc.vector.tensor_copy(
                            out=u_pp[:, kb * C:(kb + 1) * C], in_=pt)

                    softmax_pp(spool, u_pp, NMID // BLK, "smx", out=own32)
                    nc.vector.tensor_copy(out=own_hi, in_=own32)
                    nc.vector.tensor_sub(own_lo, own32, own_hi)

                    if it < ITERS - 1:
                        nc.sync.dma_start(out=ag_in, in_=own32)
                        nc.gpsimd.collective_compute(
                            "AllGather", mybir.AluOpType.bypass,
                            replica_groups=[list(range(NCORES))],
                            ins=[ag_in.opt()], outs=[ag_out[:, :]],
                        )
                        nc.sync.dma_start(
                            out=flat_pad[:, PADBLK * C:(PADBLK + GBLK) * C]
                            .rearrange("p (r j) -> p r j", r=NCORES),
                            in_=ag_out[:, :].rearrange("(r p) j -> p r j",
                                                       p=BLK))
                    else:
                        nc.sync.dma_start(out=out_dram[:, :], in_=own32)

    nc.compile()
    return nc


def _host_inputs(input_tensor, reference_tensor):
    import ml_dtypes
    bf = ml_dtypes.bfloat16

    logits = np.ascontiguousarray(
        np.asarray(input_tensor, dtype=np.float32)[0].reshape(C, N))
    ref = np.asarray(reference_tensor, dtype=np.float32)[0]  # [3, 96, 96]

    yy = (np.arange(N) // W).astype(np.float64)
    xx = (np.arange(N) % W).astype(np.float64)
    cc = ref.reshape(3, N).astype(np.float64) / 0.5
    ones = np.ones(N, np.float64)

    def hi_lo(x):
        h = np.asarray(x, np.float64).astype(bf).astype(np.float64)
        l = (np.asarray(x, np.float64) - h).astype(bf).astype(np.float64)
        return h, l

    # feature rows (G paired with H), d2 = -(dy^2)/50 - 0.5*|dc~|^2
    def feat_rows(yp):
        rows_G, rows_H = [], []

        def pair(g, h):
            rows_G.append(np.asarray(g, np.float64))
            rows_H.append(np.asarray(h, np.float64))

        g_h, g_l = hi_lo(-yp * yp / 50.0)
        pair(g_h, ones); pair(g_l, ones)
        h_h, h_l = hi_lo(yp / 25.0)
        pair(yp, h_h); pair(yp, h_l)
        h_h, h_l = hi_lo(-yp * yp / 50.0)
        pair(ones, h_h); pair(ones, h_l)
        for ch in range(3):
            cm_h, cm_l = hi_lo(cc[ch])
            pair(cm_h, cm_h); pair(cm_h, cm_l); pair(cm_l, cm_h)
        csq = -0.5 * (cc * cc).sum(axis=0)
        g_h, g_l = hi_lo(csq)
        pair(g_h, ones); pair(g_l, ones)
        pair(ones, g_h); pair(ones, g_l)
        return np.stack(rows_G), np.stack(rows_H)  # [19, N] each

    dtab = np.exp(-(np.arange(-(H + 32), H + 32) ** 2) / 50.0)
    yy_i = (np.arange(N) // W).astype(np.int64)
    xx_i = (np.arange(N) % W).astype(np.int64)

    # ipp: logits in pixel-partition layout [128, 72*5]
    ipp = np.ascontiguousarray(
        logits.reshape(C, GBLK, BLK).transpose(2, 1, 0).reshape(BLK, GBLK * C))

    # tv: 7 repeated x-Toeplitz variants [128, 7*96]
    tv = np.zeros((BLK, 7 * W), np.float16)
    for v in range(7):
        xm = (32 * (v % 3) + np.arange(BLK)) % W
        tv[:, v * W:(v + 1) * W] = dtab[
            xm[:, None] - np.arange(W)[None, :] + H + 32].astype(np.float16)

    in_maps = []
    for r in range(NCORES):
        yc = 12 * r + 6
        G_all, H_all = feat_rows(yy - yc)
        yext = np.clip(np.arange(RPC * r - 1, RPC * (r + 1) + 1), 0, H - 1)
        hpix = (yext[:, None] * W + np.arange(W)[None, :]).reshape(-1)

        g = np.zeros((NFEAT, NBLK_U * BLK), np.float64)
        g[0, :] = NEG
        for bi in range(NBLK_U):
            gb = 9 * r + BAND_LO + bi
            if 0 <= gb < GBLK:
                g[:, bi * BLK:(bi + 1) * BLK] = \
                    G_all[:, gb * BLK:(gb + 1) * BLK]
        h = H_all[:, hpix]

        sp3 = np.zeros((BLK, KW), np.float16)
        for ci, (lo_r, hi_r, ap) in enumerate(CHUNKS):
            nrows = hi_r - lo_r
            yn = yext[(lo_r + 1):(lo_r + 1) + nrows]
            for j in range(NBLKC[ci]):
                gb = 9 * r + BAND[ci][0] + j
                if not (0 <= gb < GBLK):
                    continue
                pm = np.arange(gb * BLK, (gb + 1) * BLK)
                A = 3.0 * dtab[yy_i[pm][:, None] - yn[None, :] + H + 32]
                B = dtab[xx_i[pm][:, None] - np.arange(W)[None, :] + H + 32]
                blkv = (A[:, :, None] * B[:, None, :]).reshape(BLK, ap)
                c0 = CBASE[ci] + j * ap
                sp3[:, c0:c0 + ap] = blkv.astype(np.float16)

        icn = np.ascontiguousarray(
            logits.reshape(C, H, W)[:, RPC * r:RPC * (r + 1), :]
            .reshape(C, NMID))
        in_maps.append({
            "g_feats": np.ascontiguousarray(g).astype(bf),
            "h_feats": np.ascontiguousarray(h).astype(bf),
            "sp3": sp3,
            "tv": tv,
            "inp_pp": ipp,
            "inp_cn": icn,
            "band_off": np.array([[9 * C * r]], np.uint32),
        })
    return in_maps


def _assemble(results):
    out = np.empty((C, N), np.float32)
    for r in range(NCORES):
        blk = results[r]["out_loc"].reshape(BLK, NMID // BLK, C)
        out[:, NMID * r:NMID * (r + 1)] = (
            blk.transpose(2, 1, 0).reshape(C, NMID))
    return out.reshape(1, C, H, W)


def _get_nc():
    global _CACHED_NC
    if _CACHED_NC is None:
        _CACHED_NC = _build_module()
    return _CACHED_NC


def run(input_tensor, reference_tensor, trace=False):
    from concourse.bass_utils import run_bass_kernel_spmd
    nc = _get_nc()
    in_maps = _host_inputs(input_tensor, reference_tensor)
    res = run_bass_kernel_spmd(nc, in_maps, core_ids=list(range(NCORES)),
                               trace=trace)
    return _assemble(res.results), res


def kernel(input_tensor, reference_tensor):
    out, _ = run(input_tensor, reference_tensor, trace=False)
    return out
